# revision 2
# baseline (speedup 1.0000x reference)
"""Trainium2 Bass kernel for nn_LoRAAdapter (MoE-routed LoRA adapter).

Reference computation (B=4, S=2048, D=4096, OUT=4096, E=8, R=32, topk=2):
    routing_input = x[b, eof_index[b]]                     # [B, D]
    logits = routing_input @ route_w.T + noise * (softplus(routing_input @ noise_w.T) + eps)
    gates  = scatter(softmax(top2(logits)))                # [B, E]
    shared = x @ A_w.T                                     # [B, S, R]
    out    = einsum('bsr,eor,be->bso', shared, B_w, gates) * 2.0

Strategy:
  - Routing/gating runs on host (4 tokens' worth of math) and is folded into a
    per-batch effective B matrix:  Beff2[b] = 2.0 * sum_e gates[b,e] * B_w[e].
  - Data-parallel over tokens: 8192 tokens -> 8 cores x 1024 tokens
    (core i handles batch i//2, sequence half i%2).
  - All streamed tensors are bf16 (x in, out out): halves HBM traffic vs f32
    and runs the PE at 1 cycle/row instead of 4. PSUM accumulates f32; the
    measured end-to-end max-normalized rel err is ~4e-3 (budget 2e-2).
  - The kernel is HBM-DMA-bound: 17.3 MB/core at the ~358 GB/s per-NC HBM
    limit is ~47.5 us of pipe time, vs ~27 us of PE time.  Everything is
    therefore organized around one gap-free DMA stream on the sync ring:
    all x loads are issued first (they drain at line rate, FIFO ahead of
    stores), and each out tile's store is enqueued as soon as its copies
    finish, so the ring switches from loads to stores without idling.
  - 4-block pipeline (256 tokens per block): mm1(k+1) interleaves with
    mm2(k) so the PE never sleeps long enough for the HAM clock gate to
    drop it back to 1.2 GHz, and out tiles are produced far ahead of the
    store stream (SBUF buffers all 8 MB of output).
  - PSUM->SBUF copies round-robin vector/scalar (GpSimd cannot read PSUM
    on TRN2), weighted 5:3 to match their 245:153 G elem/s rates.
"""

import numpy as np
import ml_dtypes

import concourse.bass as bass
import concourse.mybir as mybir
import concourse.tile as tile
import bass_rust
from concourse.bass_utils import run_bass_kernel_spmd

B, S, D, OUT, E, R = 4, 2048, 4096, 4096, 8, 32
TOPK = 2
NOISE_EPS = 0.01
SCALING = 2.0
N_CORES = 8
TOK = (B * S) // N_CORES          # 1024 tokens per core
BLK = 256                         # token block (mm1 moving dim)
NBLK = TOK // BLK                 # 4 blocks
DCH = D // 128                    # 32 contraction chunks of 128
G = 2                             # x DMA descriptors per block (16 chunks each)
CPG = DCH // G
OCH = OUT // 512                  # 8 output column chunks
NTILE = TOK // 128                # 8 out tiles of [128, OUT]

BF16 = ml_dtypes.bfloat16

_MAXW = 1  # this container's walrus rejects >1 sync wait per instruction


def _legalize_waits(nc):
    """Split instructions carrying >_MAXW sem waits into preceding
    same-engine nops (the kernel-tail drain waits on the whole clock).

    Two passes: nop creation appends the new instruction to the *current*
    basic block regardless of which block we are fixing, so snapshot every
    block first and rebuild each list from its own snapshot (stray appends
    then drop out naturally)."""
    snapshots = []
    for f in nc.m.functions:
        for bb in f.blocks:
            snapshots.append((bb, list(bb.instructions)))

    nops_for: dict[str, list] = {}
    for _, insts in snapshots:
        for inst in insts:
            si = inst.sync_info
            if si and si.on_wait and len(si.on_wait) > _MAXW:
                waits = list(si.on_wait)
                eng = nc.engines[inst.engine]
                extras = []
                for k in range(0, len(waits) - _MAXW, _MAXW):
                    nop = eng.nop(hint="wait_split", nofuse=True).ins
                    nop.sync_info = bass_rust.SyncInfo(
                        on_wait=waits[k : k + _MAXW], on_update=[]
                    )
                    extras.append(nop)
                si.on_wait = waits[len(waits) - _MAXW :]
                inst.sync_info = si
                nops_for[inst.name] = extras

    if not nops_for:
        return
    for bb, insts in snapshots:
        rebuilt = []
        for inst in insts:
            rebuilt.extend(nops_for.get(inst.name, ()))
            rebuilt.append(inst)
        bb.instructions = rebuilt


def build_bass():
    f32 = mybir.dt.float32
    bf16 = mybir.dt.bfloat16
    nc = bass.Bass()
    xT = nc.dram_tensor("xT", [128, NBLK, G, CPG * BLK], bf16, kind="ExternalInput")
    AT = nc.dram_tensor("AT", [128, DCH * R], bf16, kind="ExternalInput")
    B2T = nc.dram_tensor("B2T", [R, OUT], bf16, kind="ExternalInput")
    out = nc.dram_tensor("out", [TOK, OUT], bf16, kind="ExternalOutput")

    with tile.TileContext(nc) as tc:
        with (
            tc.tile_pool(name="const", bufs=1) as cpool,
            tc.tile_pool(name="xs", bufs=1) as xpool,
            tc.tile_pool(name="sh", bufs=2) as shpool,
            tc.tile_pool(name="ob", bufs=8) as opool,
            tc.tile_pool(name="psA", bufs=2, space="PSUM") as psa,
            tc.tile_pool(name="psB", bufs=6, space="PSUM") as psb,
        ):
            at_t = cpool.tile([128, DCH * R], bf16)
            nc.sync.dma_start(at_t[:], AT[:, :])
            b2_t = cpool.tile([R, OUT], bf16)

            # All loads issued upfront on the sync ring: they drain at line
            # rate FIFO-ahead of the stores, so the last x block (which
            # gates mm1(3)) arrives as early as possible, and the ring
            # never idles between the load and store phases.
            xg = {}
            for blk in range(NBLK):
                for g in range(G):
                    xt = xpool.tile([128, CPG * BLK], bf16, tag=f"xg{blk}_{g}")
                    nc.sync.dma_start(xt[:], xT[:, blk, g, :])
                    xg[blk, g] = xt
                if blk == 0:
                    # B2 after block 0's x (needed by the first mm2 at
                    # ~7us) so it doesn't delay the x chunks mm1(0) waits on
                    nc.sync.dma_start(b2_t[:], B2T[:, :])

            sh = {}

            def mm1_chunk(blk, c, ps_sh):
                g, j = divmod(c, CPG)
                nc.tensor.matmul(
                    ps_sh[:],
                    lhsT=at_t[:, c * R : (c + 1) * R],
                    rhs=xg[blk, g][:, j * BLK : (j + 1) * BLK],
                    start=(c == 0),
                    stop=(c == DCH - 1),
                )

            def sh_copy(blk, ps_sh):
                sh_sb = shpool.tile([R, BLK], bf16, tag=f"sh{blk % 2}")
                nc.vector.tensor_copy(sh_sb[:, : BLK // 2], ps_sh[:, : BLK // 2])
                nc.scalar.copy(sh_sb[:, BLK // 2 :], ps_sh[:, BLK // 2 :])
                sh[blk] = sh_sb

            ot_tiles = {}
            copies_done = {}

            def mm2_mm(tile_i, o):
                # tile_i in 0..NTILE-1 covers tokens [tile_i*128, +128);
                # block blk = tile_i // 2, within-block half t = tile_i % 2
                blk, t = divmod(tile_i, 2)
                if tile_i not in ot_tiles:
                    ot_tiles[tile_i] = opool.tile(
                        [128, OUT], bf16, tag="ot", name=f"ot{tile_i}"
                    )
                    copies_done[tile_i] = 0
                ot = ot_tiles[tile_i]
                ps_o = psb.tile([128, 512], f32, tag="ps_o")
                nc.tensor.matmul(
                    ps_o[:],
                    lhsT=sh[blk][:, t * 128 : (t + 1) * 128],
                    rhs=b2_t[:, o * 512 : (o + 1) * 512],
                    start=True,
                    stop=True,
                )
                oc = ot[:, o * 512 : (o + 1) * 512]
                # vector is ~1.6x faster than scalar at PSUM->SBUF: give it
                # chunks {0,1,2,4,5} and scalar {3,6,7} (5:3 split)
                if o in (3, 6, 7):
                    nc.scalar.copy(oc, ps_o[:])
                else:
                    nc.vector.tensor_copy(oc, ps_o[:])
                copies_done[tile_i] += 1
                if copies_done[tile_i] == OCH:
                    t0 = tile_i * 128
                    nc.sync.dma_start(out[t0 : t0 + 128, :], ot[:, :])

            # ---- PE stream ----
            # mm1(0) alone (nothing to overlap yet), then each block's 16
            # mm2 matmuls interleave 1:2 with the next block's 32 mm1
            # chunks; block 3's mm2s run at the tail.
            ps = psa.tile([R, BLK], f32, tag="ps_sh")
            for c in range(DCH):
                mm1_chunk(0, c, ps)
            sh_copy(0, ps)

            for blk in range(NBLK):
                if blk + 1 < NBLK:
                    mm2s = [(blk * 2 + t, o) for t in range(2) for o in range(OCH)]
                    ps = psa.tile([R, BLK], f32, tag="ps_sh")
                    for c in range(DCH):
                        mm1_chunk(blk + 1, c, ps)
                        if c % 2 == 1:
                            mm2_mm(*mm2s[c // 2])
                    sh_copy(blk + 1, ps)
                else:
                    for t in range(2):
                        for o in range(OCH):
                            mm2_mm(blk * 2 + t, o)
    _legalize_waits(nc)
    return nc


_NC_CACHE = {}


def _get_nc():
    if "nc" not in _NC_CACHE:
        _NC_CACHE["nc"] = build_bass()
    return _NC_CACHE["nc"]


def _softplus(v):
    return np.logaddexp(0.0, v)


def _host_prep(x, eof_index, noise, A_w, B_w, route_w, noise_w):
    """Routing + gating on host; returns per-core input maps."""
    x = np.asarray(x, dtype=np.float32)
    eof = np.asarray(eof_index).astype(np.int64)
    noise = np.asarray(noise, dtype=np.float32)
    A_w = np.asarray(A_w, dtype=np.float32)
    B_w = np.asarray(B_w, dtype=np.float32)
    route_w = np.asarray(route_w, dtype=np.float32)
    noise_w = np.asarray(noise_w, dtype=np.float32)

    rows = np.arange(B)
    routing_input = x[rows, eof]                                  # [B, D]
    clean = routing_input @ route_w.T                             # [B, E]
    stddev = _softplus(routing_input @ noise_w.T) + NOISE_EPS
    logits = clean + noise * stddev
    top_idx = np.argsort(-logits, axis=-1, kind="stable")[:, :TOPK]
    top_vals = np.take_along_axis(logits, top_idx, axis=-1)
    m = top_vals.max(axis=-1, keepdims=True)
    ex = np.exp(top_vals - m)
    top_gates = (ex / ex.sum(axis=-1, keepdims=True)).astype(np.float32)
    gates = np.zeros((B, E), np.float32)
    np.put_along_axis(gates, top_idx, top_gates, axis=-1)

    # Beff2[b] = SCALING * sum_e gates[b,e] * B_w[e]   -> [B, OUT, R]
    beff2 = SCALING * np.einsum("be,eor->bor", gates, B_w)

    # AT layout [128, DCH, R]: at[p, c, r] = A_w[r, c*128+p]
    at = np.ascontiguousarray(
        A_w.reshape(R, DCH, 128).transpose(2, 1, 0).reshape(128, DCH * R)
    ).astype(BF16)

    in_maps = []
    for i in range(N_CORES):
        b = i * TOK // S
        t0 = i * TOK - b * S
        xc = x[b, t0 : t0 + TOK, :]                               # [TOK, D]
        # [128, NBLK, DCH, BLK]: xT[p, blk, c, t] = xc[blk*BLK+t, c*128+p]
        xT_i = np.ascontiguousarray(
            xc.reshape(NBLK, BLK, DCH, 128).transpose(3, 0, 2, 1)
        ).astype(BF16).reshape(128, NBLK, G, CPG * BLK)
        b2t_i = np.ascontiguousarray(beff2[b].T).astype(BF16)     # [R, OUT]
        in_maps.append({"xT": xT_i, "AT": at, "B2T": b2t_i})
    return in_maps


def _run(in_maps, trace=False, **kw):
    nc = _get_nc()
    return run_bass_kernel_spmd(
        nc, in_maps, core_ids=list(range(N_CORES)), trace=trace, **kw
    )


def kernel(x, eof_index, noise, A_w, B_w, route_w, noise_w):
    in_maps = _host_prep(x, eof_index, noise, A_w, B_w, route_w, noise_w)
    res = _run(in_maps)
    out = np.empty((B, S, OUT), np.float32)
    for i in range(N_CORES):
        b = i * TOK // S
        t0 = i * TOK - b * S
        out[b, t0 : t0 + TOK, :] = np.asarray(res.results[i]["out"]).astype(
            np.float32
        )
    return out
